# revision 10
# baseline (speedup 1.0000x reference)
"""Entmax-1.5 multi-head attention on 8 Trainium2 NeuronCores.

Head-sharded tensor parallelism: each of the 8 cores owns 2 of the 16 heads.
Per core: QKV projections (column-sliced weights), scores = qk^T/sqrt(Dh),
entmax-1.5 via Newton iteration on the threshold equation
    f(tau) = sum_j relu(Xa_j - tau)^2 - 1 = 0   (alpha=1.5 => exponent 2),
which matches the reference's 50-step bisection root to fp32 precision,
then attn @ v, and a row-sliced output projection producing partial sums
that the host reduces.

Engine split per 128x1024 score tile: ACT does relu(x - tau) with running
sum (s = f'(tau)/-2), DVE does the square-accumulate (q = sum r^2) plus the
Newton-step smalls, PE does all matmuls/transposes, HWDGE DMAs move data.
"""

import numpy as np
from contextlib import ExitStack

import concourse.bass as bass
import concourse.tile as tile
from concourse import mybir
from concourse.vector_clock import ScopedClock

# ---------------------------------------------------------------------------
# Workaround for this container's walrus: TPB instructions encode only ONE
# sync wait. Tile emits multi-wait sync_info; split extras onto same-engine
# InstNoOp carriers after scheduling.
# ---------------------------------------------------------------------------
_PATCHED = False


def _install_tile_patch():
    global _PATCHED
    if _PATCHED:
        return
    _PATCHED = True
    _orig_exit = tile.TileContext.__exit__
    ctr = [0]

    def _split_multi_waits(nc):
        for fn in nc.m.functions:
            for blk in fn.blocks:
                insts = blk.instructions
                out = []
                changed = False
                for ins in insts:
                    si = getattr(ins, "sync_info", None)
                    waits = list(si.on_wait) if (si and si.on_wait) else []
                    if len(waits) > 1:
                        changed = True
                        for w in waits[:-1]:
                            ctr[0] += 1
                            nop = mybir.InstNoOp(
                                name=f"waitsplit-{ctr[0]}", ins=[], outs=[]
                            )
                            nop.engine = ins.engine
                            nop.sync_info = mybir.SyncInfo(
                                on_wait=[w], on_update=[]
                            )
                            out.append(nop)
                        del si.on_wait[:-1]
                    out.append(ins)
                if changed:
                    blk.instructions = out

    def _drain_and_barrier_split(self, tick_clock, wait_clock):
        nc = self.nc
        drain_inst = nc.sync.drain()
        wait_clock.add_sem_waits(
            drain_inst.ins, ScopedClock({None: tick_clock.global_clock})
        )
        nc.all_engine_barrier()
        assert self.sems is not None
        popped = nc._tile_sem_poison_stack.pop()
        assert popped is self._sem_poison
        nc.clear_and_free_semaphores(list(self.sems.allocated().values()))
        nc.all_engine_barrier()

    def _patched_exit(self, exc_type, exc_val, exc_tb):
        r = _orig_exit(self, exc_type, exc_val, exc_tb)
        if exc_type is None:
            _split_multi_waits(self.nc)
        return r

    tile.TileContext._drain_and_barrier = _drain_and_barrier_split
    tile.TileContext.__exit__ = _patched_exit


# ---------------------------------------------------------------------------
# Problem constants (hardcoded per harness contract)
# ---------------------------------------------------------------------------
B, N, C = 4, 1024, 1024
H, DH = 16, 64
NCORES = 8
HPC = H // NCORES          # heads per core = 2
NT = N // 128              # row tiles per pair = 8
R_NEWTON = 8               # Newton rounds (validated: max|err| ~2.4e-7 vs ref)
SCALE = 0.0625             # (1/sqrt(DH)) * (alpha-1) = 0.125 * 0.5
F32 = mybir.dt.float32
ALU = mybir.AluOpType
AF = mybir.ActivationFunctionType


def _build_kernel():
    _install_tile_patch()
    from concourse.masks import make_identity

    nc = bass.Bass()
    x_d = nc.dram_tensor("x", (B, N, C), F32, kind="ExternalInput")
    wq_d = nc.dram_tensor("wq", (C, HPC * DH), F32, kind="ExternalInput")
    wk_d = nc.dram_tensor("wk", (C, HPC * DH), F32, kind="ExternalInput")
    wv_d = nc.dram_tensor("wv", (C, HPC * DH), F32, kind="ExternalInput")
    wo_d = nc.dram_tensor("wo", (HPC * DH, C), F32, kind="ExternalInput")
    bq_d = nc.dram_tensor("bq", (HPC * DH, 1), F32, kind="ExternalInput")
    bk_d = nc.dram_tensor("bk", (HPC * DH, 1), F32, kind="ExternalInput")
    bv_d = nc.dram_tensor("bv", (HPC * DH, 1), F32, kind="ExternalInput")
    attn_d = nc.dram_tensor("attn_out", (B, HPC, N, N), F32, kind="ExternalOutput")
    out_d = nc.dram_tensor("out_part", (B, N, C), F32, kind="ExternalOutput")

    with tile.TileContext(nc) as tc, ExitStack() as ctx:
        singles = ctx.enter_context(tc.tile_pool(name="singles", bufs=1))
        xload = ctx.enter_context(tc.tile_pool(name="xload", bufs=3))
        xtp_pool = ctx.enter_context(tc.tile_pool(name="xtsb", bufs=1))
        qkv = ctx.enter_context(tc.tile_pool(name="qkv", bufs=1))
        xa_pool = ctx.enter_context(tc.tile_pool(name="xa", bufs=10))
        rpool = ctx.enter_context(tc.tile_pool(name="rt", bufs=3))
        atp = ctx.enter_context(tc.tile_pool(name="atp", bufs=2))
        smalls = ctx.enter_context(tc.tile_pool(name="smalls", bufs=2))
        opool = ctx.enter_context(tc.tile_pool(name="oev", bufs=2))
        psum = ctx.enter_context(tc.tile_pool(name="ps", bufs=2, space="PSUM"))
        psav = ctx.enter_context(tc.tile_pool(name="psav", bufs=1, space="PSUM"))

        # ---- weights / constants ----
        ident = singles.tile([128, 128], F32)
        make_identity(nc, ident)
        w_sb = {}
        for nm, dram in (("wq", wq_d), ("wk", wk_d), ("wv", wv_d)):
            t = singles.tile([128, 8, 128], F32, tag=nm)
            nc.sync.dma_start(out=t, in_=dram.rearrange("(k p) d -> p k d", p=128))
            w_sb[nm] = t
        wo_h = []
        for hh in range(HPC):
            t = singles.tile([64, 1024], F32, tag=f"wo{hh}")
            nc.sync.dma_start(out=t, in_=wo_d[hh * 64:(hh + 1) * 64, :])
            wo_h.append(t)
        bias_sb = {}
        for nm, dram in (("bq", bq_d), ("bk", bk_d)):
            # head hh's bias staged on partitions [hh*64, hh*64+64) so it can
            # ride PSUM evacuations that stay on native partitions
            t = singles.tile([128, 1], F32, tag=f"{nm}")
            for hh in range(HPC):
                nc.sync.dma_start(
                    out=t[hh * 64:(hh + 1) * 64, :],
                    in_=dram[hh * 64:(hh + 1) * 64, :],
                )
            bias_sb[nm] = t
        bv_h = []
        for hh in range(HPC):
            t = singles.tile([64, 1], F32, tag=f"bv{hh}")
            nc.sync.dma_start(out=t, in_=bv_d[hh * 64:(hh + 1) * 64, :])
            bv_h.append(t)
        junkD = singles.tile([128, 1024], F32, tag="junkD")

        # per-batch qkv projections; x^T built per n-half to bound SBUF
        def project_batch(b):
            heads = {}
            for nm in ("wq", "wk"):
                for hh in range(HPC):
                    head_t = qkv.tile([64, 1024], F32, tag=f"{nm}T{hh}")
                    heads[(nm, hh)] = head_t
            vt = qkv.tile([128, 8, 128], F32, tag="v")
            for nh in range(2):
                xT = xtp_pool.tile([128, 8, 512], F32, tag="xT")
                for ln in range(4):
                    nt = nh * 4 + ln
                    xt_in = xload.tile([128, 1024], F32, tag="xin")
                    nc.sync.dma_start(
                        out=xt_in, in_=x_d[b, nt * 128:(nt + 1) * 128, :]
                    )
                    for cg in range(2):       # two groups of 4 c-chunks
                        pt = psum.tile([128, 4, 128], F32, tag="half")
                        for j in range(4):
                            ct = cg * 4 + j
                            nc.tensor.transpose(
                                pt[:, j, :], xt_in[:, ct * 128:(ct + 1) * 128],
                                ident,
                            )
                        nc.scalar.copy(
                            out=xT[:, cg * 4:(cg + 1) * 4,
                                   ln * 128:(ln + 1) * 128],
                            in_=pt,
                        )
                for nm, bias_nm in (("wq", "bq"), ("wk", "bk")):
                    qp = psum.tile([128, 512], F32, tag="half")
                    for ct in range(8):
                        nc.tensor.matmul(
                            qp,
                            w_sb[nm][:, ct, :],
                            xT[:, ct, :],
                            start=(ct == 0),
                            stop=(ct == 7),
                        )
                    # evacuate per head (keep partitions 0:64 native)
                    nc.scalar.activation(
                        out=heads[(nm, 0)][:, nh * 512:(nh + 1) * 512],
                        in_=qp[0:64, :], func=AF.Identity,
                        bias=bias_sb[bias_nm][0:64, 0:1], scale=1.0,
                    )
                    # h1 lives on partitions 64:128; shift to 0:64 via DMA
                    tmp = opool.tile([128, 512], F32, tag="shift")
                    nc.scalar.activation(
                        out=tmp[64:128, :], in_=qp[64:128, :], func=AF.Identity,
                        bias=bias_sb[bias_nm][64:128, 0:1], scale=1.0,
                    )
                    nc.sync.dma_start(
                        out=heads[(nm, 1)][:, nh * 512:(nh + 1) * 512],
                        in_=tmp[64:128, :],
                    )
                for lm in range(4):
                    mt = nh * 4 + lm
                    vp = psum.tile([128, 128], F32, tag="half")
                    for ct in range(8):
                        nc.tensor.matmul(
                            vp,
                            xT[:, ct, lm * 128:(lm + 1) * 128],
                            w_sb["wv"][:, ct, :],
                            start=(ct == 0),
                            stop=(ct == 7),
                        )
                    nc.scalar.copy(out=vt[:, mt, :], in_=vp)
            return ([heads[("wq", 0)], heads[("wq", 1)]],
                    [heads[("wk", 0)], heads[("wk", 1)]], vt)

        outT_h = []
        for hh in range(HPC):
            outT_t = singles.tile([64, B, 1024], F32, tag=f"outT{hh}")
            outT_h.append(outT_t)

        for b in range(B):
            qTh, kTh, vt = project_batch(b)
            for hh in range(HPC):
                qT, kT = qTh[hh], kTh[hh]
                dlo = hh * 64
                # ---- scores -> Xa tiles (scaled), row maxes ----
                xa_tiles = []
                mx2 = smalls.tile([128, NT, 2], F32, tag="mx2")
                for nt in range(NT):
                    sp = psum.tile([128, 1024], F32, tag="wide")
                    for mh in range(2):
                        nc.tensor.matmul(
                            sp[:, mh * 512:(mh + 1) * 512],
                            qT[:, nt * 128:(nt + 1) * 128],
                            kT[:, mh * 512:(mh + 1) * 512],
                            start=True,
                            stop=True,
                        )
                    xa = xa_pool.tile([128, 1024], F32, tag="xa")
                    for mh in range(2):
                        nc.vector.tensor_scalar(
                            out=xa[:, mh * 512:(mh + 1) * 512],
                            in0=sp[:, mh * 512:(mh + 1) * 512],
                            scalar1=SCALE, scalar2=None,
                            op0=ALU.mult, op1=ALU.max,
                            accum_out=mx2[:, nt, mh:mh + 1],
                        )
                    xa_tiles.append(xa)
                # negtau = 1 - rowmax
                negtau = smalls.tile([128, NT], F32, tag="negtau")
                nc.vector.tensor_max(negtau, mx2[:, :, 0], mx2[:, :, 1])
                nc.vector.tensor_scalar(
                    out=negtau, in0=negtau, scalar1=-1.0, scalar2=1.0,
                    op0=ALU.mult, op1=ALU.add,
                )
                # ---- Newton rounds ----
                s_buf = smalls.tile([128, NT], F32, tag="sbuf")
                q_buf = smalls.tile([128, NT], F32, tag="qbuf")
                tmp_a = smalls.tile([128, NT], F32, tag="tmpa")
                tmp_b = smalls.tile([128, NT], F32, tag="tmpb")
                for it in range(R_NEWTON):
                    r_tiles = []
                    for nt in range(NT):
                        r = rpool.tile([128, 1024], F32, tag="r")
                        nc.scalar.activation(
                            out=r, in_=xa_tiles[nt], func=AF.Relu,
                            bias=negtau[:, nt:nt + 1], scale=1.0,
                            accum_out=s_buf[:, nt:nt + 1],
                        )
                        nc.vector.scalar_tensor_tensor(
                            out=junkD, in0=r, scalar=1.0, in1=r,
                            op0=ALU.mult, op1=ALU.mult,
                            accum_out=q_buf[:, nt:nt + 1],
                        )
                        r_tiles.append(r)
                    # newton step: negtau -= (q-1) / (2s)
                    nc.vector.tensor_add(tmp_a, s_buf, s_buf)
                    nc.vector.reciprocal(out=tmp_a, in_=tmp_a)
                    nc.vector.tensor_scalar(
                        out=tmp_b, in0=q_buf, scalar1=-1.0, scalar2=None,
                        op0=ALU.add,
                    )
                    nc.vector.tensor_mul(tmp_b, tmp_b, tmp_a)
                    nc.vector.tensor_sub(negtau, negtau, tmp_b)
                # ---- final p, normalize ----
                S_buf = smalls.tile([128, NT], F32, tag="Sbuf")
                for nt in range(NT):
                    r = rpool.tile([128, 1024], F32, tag="r")
                    nc.scalar.activation(
                        out=r, in_=xa_tiles[nt], func=AF.Relu,
                        bias=negtau[:, nt:nt + 1], scale=1.0,
                    )
                    nc.vector.scalar_tensor_tensor(
                        out=xa_tiles[nt], in0=r, scalar=1.0, in1=r,
                        op0=ALU.mult, op1=ALU.mult,
                        accum_out=S_buf[:, nt:nt + 1],
                    )
                recipS = smalls.tile([128, NT], F32, tag="recipS")
                nc.vector.reciprocal(out=recipS, in_=S_buf)
                for nt in range(NT):
                    nc.scalar.activation(
                        out=xa_tiles[nt], in_=xa_tiles[nt], func=AF.Copy,
                        bias=0.0, scale=recipS[:, nt:nt + 1],
                    )
                    nc.sync.dma_start(
                        out=attn_d[b, hh, nt * 128:(nt + 1) * 128, :],
                        in_=xa_tiles[nt],
                    )
                # ---- attn @ v (transpose attn per m-chunk, accumulate) ----
                avp = psav.tile([64, 1024], F32, tag="avp")
                for mt in range(8):
                    attnT = atp.tile([128, 1024], F32, tag="attnT")
                    for ng in range(2):
                        tp = psum.tile([128, 4, 128], F32, tag="half")
                        for j in range(4):
                            nt = ng * 4 + j
                            nc.tensor.transpose(
                                tp[:, j, :],
                                xa_tiles[nt][:, mt * 128:(mt + 1) * 128],
                                ident,
                            )
                        nc.vector.tensor_copy(
                            out=attnT[:, ng * 512:(ng + 1) * 512], in_=tp
                        )
                    for nh in range(2):
                        nc.tensor.matmul(
                            avp[:, nh * 512:(nh + 1) * 512],
                            vt[:, mt, dlo:dlo + 64],
                            attnT[:, nh * 512:(nh + 1) * 512],
                            start=(mt == 0),
                            stop=(mt == 7),
                        )
                nc.scalar.activation(
                    out=outT_h[hh][:, b, :], in_=avp, func=AF.Identity,
                    bias=bv_h[hh][:, 0:1], scale=1.0,
                )
            # ---- output projection for batch b (both heads accumulate) ----
            for nt in range(NT):
                op = psum.tile([128, 1024], F32, tag="wide")
                for co in range(2):
                    for hh in range(HPC):
                        nc.tensor.matmul(
                            op[:, co * 512:(co + 1) * 512],
                            outT_h[hh][:, b, nt * 128:(nt + 1) * 128],
                            wo_h[hh][:, co * 512:(co + 1) * 512],
                            start=(hh == 0),
                            stop=(hh == HPC - 1),
                        )
                oe = opool.tile([128, 1024], F32, tag="oev")
                nc.scalar.copy(out=oe, in_=op)
                nc.sync.dma_start(
                    out=out_d[b, nt * 128:(nt + 1) * 128, :], in_=oe
                )
    return nc


_NC_CACHE = None


def _get_nc():
    global _NC_CACHE
    if _NC_CACHE is None:
        _NC_CACHE = _build_kernel()
    return _NC_CACHE


def kernel(x, Wq, bq, Wk, bk, Wv, bv, Wo, bo, _profile=None):
    from concourse.bass_utils import run_bass_kernel_spmd

    x = np.ascontiguousarray(np.asarray(x, dtype=np.float32))
    nc = _get_nc()
    in_maps = []
    for c in range(NCORES):
        cols = slice(c * HPC * DH, (c + 1) * HPC * DH)
        in_maps.append({
            "x": x,
            "wq": np.ascontiguousarray(np.asarray(Wq, np.float32)[:, cols]),
            "wk": np.ascontiguousarray(np.asarray(Wk, np.float32)[:, cols]),
            "wv": np.ascontiguousarray(np.asarray(Wv, np.float32)[:, cols]),
            "wo": np.ascontiguousarray(np.asarray(Wo, np.float32)[cols, :]),
            "bq": np.ascontiguousarray(np.asarray(bq, np.float32)[cols, None]),
            "bk": np.ascontiguousarray(np.asarray(bk, np.float32)[cols, None]),
            "bv": np.ascontiguousarray(np.asarray(bv, np.float32)[cols, None]),
        })
    res = run_bass_kernel_spmd(nc, in_maps, core_ids=list(range(NCORES)))
    attn = np.empty((B, H, N, N), np.float32)
    out = np.zeros((B, N, C), np.float64)
    for c in range(NCORES):
        attn[:, c * HPC:(c + 1) * HPC] = res.results[c]["attn_out"]
        out += res.results[c]["out_part"]
    out = (out + np.asarray(bo, np.float64)).astype(np.float32)
    kernel._last_results = res
    return out, attn


# revision 11
# speedup vs baseline: 1.0208x; 1.0208x over previous
"""Entmax-1.5 multi-head attention on 8 Trainium2 NeuronCores.

Head-sharded tensor parallelism: each of the 8 cores owns 2 of the 16 heads.
Per core: QKV projections (column-sliced weights), scores = qk^T/sqrt(Dh),
entmax-1.5 via Newton iteration on the threshold equation
    f(tau) = sum_j relu(Xa_j - tau)^2 - 1 = 0   (alpha=1.5 => exponent 2),
which matches the reference's 50-step bisection root to fp32 precision,
then attn @ v, and a row-sliced output projection producing partial sums
that the host reduces.

Engine split per 128x1024 score tile: ACT does relu(x - tau) with running
sum (s = f'(tau)/-2), DVE does the square-accumulate (q = sum r^2) plus the
Newton-step smalls, PE does all matmuls/transposes, HWDGE DMAs move data.
"""

import numpy as np
from contextlib import ExitStack

import concourse.bass as bass
import concourse.tile as tile
from concourse import mybir
from concourse.vector_clock import ScopedClock

# ---------------------------------------------------------------------------
# Workaround for this container's walrus: TPB instructions encode only ONE
# sync wait. Tile emits multi-wait sync_info; split extras onto same-engine
# InstNoOp carriers after scheduling.
# ---------------------------------------------------------------------------
_PATCHED = False


def _install_tile_patch():
    global _PATCHED
    if _PATCHED:
        return
    _PATCHED = True
    _orig_exit = tile.TileContext.__exit__
    ctr = [0]

    def _split_multi_waits(nc):
        for fn in nc.m.functions:
            for blk in fn.blocks:
                insts = blk.instructions
                out = []
                changed = False
                for ins in insts:
                    si = getattr(ins, "sync_info", None)
                    waits = list(si.on_wait) if (si and si.on_wait) else []
                    if len(waits) > 1:
                        changed = True
                        for w in waits[:-1]:
                            ctr[0] += 1
                            nop = mybir.InstNoOp(
                                name=f"waitsplit-{ctr[0]}", ins=[], outs=[]
                            )
                            nop.engine = ins.engine
                            nop.sync_info = mybir.SyncInfo(
                                on_wait=[w], on_update=[]
                            )
                            out.append(nop)
                        del si.on_wait[:-1]
                    out.append(ins)
                if changed:
                    blk.instructions = out

    def _drain_and_barrier_split(self, tick_clock, wait_clock):
        nc = self.nc
        drain_inst = nc.sync.drain()
        wait_clock.add_sem_waits(
            drain_inst.ins, ScopedClock({None: tick_clock.global_clock})
        )
        nc.all_engine_barrier()
        assert self.sems is not None
        popped = nc._tile_sem_poison_stack.pop()
        assert popped is self._sem_poison
        nc.clear_and_free_semaphores(list(self.sems.allocated().values()))
        nc.all_engine_barrier()

    def _patched_exit(self, exc_type, exc_val, exc_tb):
        r = _orig_exit(self, exc_type, exc_val, exc_tb)
        if exc_type is None:
            _split_multi_waits(self.nc)
        return r

    tile.TileContext._drain_and_barrier = _drain_and_barrier_split
    tile.TileContext.__exit__ = _patched_exit


# ---------------------------------------------------------------------------
# Problem constants (hardcoded per harness contract)
# ---------------------------------------------------------------------------
B, N, C = 4, 1024, 1024
H, DH = 16, 64
NCORES = 8
HPC = H // NCORES          # heads per core = 2
NT = N // 128              # row tiles per pair = 8
R_NEWTON = 8               # Newton rounds (validated: max|err| ~2.4e-7 vs ref)
SCALE = 0.0625             # (1/sqrt(DH)) * (alpha-1) = 0.125 * 0.5
F32 = mybir.dt.float32
ALU = mybir.AluOpType
AF = mybir.ActivationFunctionType


def _build_kernel():
    _install_tile_patch()
    from concourse.masks import make_identity

    nc = bass.Bass()
    x_d = nc.dram_tensor("x", (B, N, C), F32, kind="ExternalInput")
    wq_d = nc.dram_tensor("wq", (C, HPC * DH), F32, kind="ExternalInput")
    wk_d = nc.dram_tensor("wk", (C, HPC * DH), F32, kind="ExternalInput")
    wv_d = nc.dram_tensor("wv", (C, HPC * DH), F32, kind="ExternalInput")
    wo_d = nc.dram_tensor("wo", (HPC * DH, C), F32, kind="ExternalInput")
    bq_d = nc.dram_tensor("bq", (HPC * DH, 1), F32, kind="ExternalInput")
    bk_d = nc.dram_tensor("bk", (HPC * DH, 1), F32, kind="ExternalInput")
    bv_d = nc.dram_tensor("bv", (HPC * DH, 1), F32, kind="ExternalInput")
    attn_d = nc.dram_tensor("attn_out", (B, HPC, N, N), F32, kind="ExternalOutput")
    out_d = nc.dram_tensor("out_part", (B, N, C), F32, kind="ExternalOutput")

    with tile.TileContext(nc) as tc, ExitStack() as ctx:
        singles = ctx.enter_context(tc.tile_pool(name="singles", bufs=1))
        xload = ctx.enter_context(tc.tile_pool(name="xload", bufs=3))
        xtp_pool = ctx.enter_context(tc.tile_pool(name="xtsb", bufs=1))
        qkv = ctx.enter_context(tc.tile_pool(name="qkv", bufs=1))
        xa_pool = ctx.enter_context(tc.tile_pool(name="xa", bufs=10))
        rpool = ctx.enter_context(tc.tile_pool(name="rt", bufs=3))
        atp = ctx.enter_context(tc.tile_pool(name="atp", bufs=2))
        smalls = ctx.enter_context(tc.tile_pool(name="smalls", bufs=2))
        opool = ctx.enter_context(tc.tile_pool(name="oev", bufs=2))
        psum = ctx.enter_context(tc.tile_pool(name="ps", bufs=2, space="PSUM"))
        psav = ctx.enter_context(tc.tile_pool(name="psav", bufs=1, space="PSUM"))

        # ---- weights / constants ----
        ident = singles.tile([128, 128], F32)
        make_identity(nc, ident)
        w_sb = {}
        for nm, dram in (("wq", wq_d), ("wk", wk_d), ("wv", wv_d)):
            t = singles.tile([128, 8, 128], F32, tag=nm)
            nc.sync.dma_start(out=t, in_=dram.rearrange("(k p) d -> p k d", p=128))
            w_sb[nm] = t
        wo_h = []
        for hh in range(HPC):
            t = singles.tile([64, 1024], F32, tag=f"wo{hh}")
            nc.sync.dma_start(out=t, in_=wo_d[hh * 64:(hh + 1) * 64, :])
            wo_h.append(t)
        bias_sb = {}
        for nm, dram in (("bq", bq_d), ("bk", bk_d)):
            # head hh's bias staged on partitions [hh*64, hh*64+64) so it can
            # ride PSUM evacuations that stay on native partitions
            t = singles.tile([128, 1], F32, tag=f"{nm}")
            for hh in range(HPC):
                nc.sync.dma_start(
                    out=t[hh * 64:(hh + 1) * 64, :],
                    in_=dram[hh * 64:(hh + 1) * 64, :],
                )
            bias_sb[nm] = t
        bv_h = []
        for hh in range(HPC):
            t = singles.tile([64, 1], F32, tag=f"bv{hh}")
            nc.sync.dma_start(out=t, in_=bv_d[hh * 64:(hh + 1) * 64, :])
            bv_h.append(t)
        junkD = singles.tile([128, 1024], F32, tag="junkD")

        # per-batch qkv projections; x^T built per n-half to bound SBUF
        def project_batch(b):
            heads = {}
            for nm in ("wq", "wk"):
                for hh in range(HPC):
                    head_t = qkv.tile([64, 1024], F32, tag=f"{nm}T{hh}")
                    heads[(nm, hh)] = head_t
            vt = qkv.tile([128, 8, 128], F32, tag="v")
            for nh in range(2):
                xT = xtp_pool.tile([128, 8, 512], F32, tag="xT")
                for ln in range(4):
                    nt = nh * 4 + ln
                    xt_in = xload.tile([128, 1024], F32, tag="xin")
                    nc.sync.dma_start(
                        out=xt_in, in_=x_d[b, nt * 128:(nt + 1) * 128, :]
                    )
                    for cg in range(2):       # two groups of 4 c-chunks
                        pt = psum.tile([128, 4, 128], F32, tag="half")
                        for j in range(4):
                            ct = cg * 4 + j
                            nc.tensor.transpose(
                                pt[:, j, :], xt_in[:, ct * 128:(ct + 1) * 128],
                                ident,
                            )
                        nc.scalar.copy(
                            out=xT[:, cg * 4:(cg + 1) * 4,
                                   ln * 128:(ln + 1) * 128],
                            in_=pt,
                        )
                for nm, bias_nm in (("wq", "bq"), ("wk", "bk")):
                    qp = psum.tile([128, 512], F32, tag="half")
                    for ct in range(8):
                        nc.tensor.matmul(
                            qp,
                            w_sb[nm][:, ct, :],
                            xT[:, ct, :],
                            start=(ct == 0),
                            stop=(ct == 7),
                        )
                    # evacuate per head (keep partitions 0:64 native)
                    nc.scalar.activation(
                        out=heads[(nm, 0)][:, nh * 512:(nh + 1) * 512],
                        in_=qp[0:64, :], func=AF.Identity,
                        bias=bias_sb[bias_nm][0:64, 0:1], scale=1.0,
                    )
                    # h1 lives on partitions 64:128; shift to 0:64 via DMA
                    tmp = opool.tile([128, 512], F32, tag="shift")
                    nc.scalar.activation(
                        out=tmp[64:128, :], in_=qp[64:128, :], func=AF.Identity,
                        bias=bias_sb[bias_nm][64:128, 0:1], scale=1.0,
                    )
                    nc.sync.dma_start(
                        out=heads[(nm, 1)][:, nh * 512:(nh + 1) * 512],
                        in_=tmp[64:128, :],
                    )
                for lm in range(4):
                    mt = nh * 4 + lm
                    vp = psum.tile([128, 128], F32, tag="half")
                    for ct in range(8):
                        nc.tensor.matmul(
                            vp,
                            xT[:, ct, lm * 128:(lm + 1) * 128],
                            w_sb["wv"][:, ct, :],
                            start=(ct == 0),
                            stop=(ct == 7),
                        )
                    nc.scalar.copy(out=vt[:, mt, :], in_=vp)
            return ([heads[("wq", 0)], heads[("wq", 1)]],
                    [heads[("wk", 0)], heads[("wk", 1)]], vt)

        outT_h = []
        for hh in range(HPC):
            outT_t = singles.tile([64, B, 1024], F32, tag=f"outT{hh}")
            outT_h.append(outT_t)

        for b in range(B):
            qTh, kTh, vt = project_batch(b)
            for hh in range(HPC):
                qT, kT = qTh[hh], kTh[hh]
                dlo = hh * 64
                # ---- scores -> Xa tiles (scaled), row maxes ----
                xa_tiles = []
                mx2 = smalls.tile([128, NT, 2], F32, tag="mx2")
                for nt in range(NT):
                    sp = psum.tile([128, 1024], F32, tag="wide")
                    for mh in range(2):
                        nc.tensor.matmul(
                            sp[:, mh * 512:(mh + 1) * 512],
                            qT[:, nt * 128:(nt + 1) * 128],
                            kT[:, mh * 512:(mh + 1) * 512],
                            start=True,
                            stop=True,
                        )
                    xa = xa_pool.tile([128, 1024], F32, tag="xa")
                    for mh in range(2):
                        nc.vector.tensor_scalar(
                            out=xa[:, mh * 512:(mh + 1) * 512],
                            in0=sp[:, mh * 512:(mh + 1) * 512],
                            scalar1=SCALE, scalar2=None,
                            op0=ALU.mult, op1=ALU.max,
                            accum_out=mx2[:, nt, mh:mh + 1],
                        )
                    xa_tiles.append(xa)
                # negtau = 1 - rowmax
                negtau = smalls.tile([128, NT], F32, tag="negtau")
                nc.vector.tensor_max(negtau, mx2[:, :, 0], mx2[:, :, 1])
                nc.vector.tensor_scalar(
                    out=negtau, in0=negtau, scalar1=-1.0, scalar2=1.0,
                    op0=ALU.mult, op1=ALU.add,
                )
                # ---- Newton rounds ----
                s_buf = smalls.tile([128, NT], F32, tag="sbuf")
                q_buf = smalls.tile([128, NT], F32, tag="qbuf")
                tmp_a = smalls.tile([128, NT], F32, tag="tmpa")
                tmp_b = smalls.tile([128, NT], F32, tag="tmpb")
                for it in range(R_NEWTON):
                    r_tiles = []
                    for nt in range(NT):
                        r = rpool.tile([128, 1024], F32, tag="r")
                        nc.scalar.activation(
                            out=r, in_=xa_tiles[nt], func=AF.Relu,
                            bias=negtau[:, nt:nt + 1], scale=1.0,
                            accum_out=s_buf[:, nt:nt + 1],
                        )
                        nc.vector.scalar_tensor_tensor(
                            out=junkD, in0=r, scalar=1.0, in1=r,
                            op0=ALU.mult, op1=ALU.mult,
                            accum_out=q_buf[:, nt:nt + 1],
                        )
                        r_tiles.append(r)
                    # newton step: negtau -= (q-1) / (2s)
                    nc.vector.reciprocal(out=tmp_a, in_=s_buf)
                    nc.vector.scalar_tensor_tensor(
                        out=tmp_b, in0=q_buf, scalar=-1.0, in1=tmp_a,
                        op0=ALU.add, op1=ALU.mult,
                    )
                    nc.vector.scalar_tensor_tensor(
                        out=negtau, in0=tmp_b, scalar=-0.5, in1=negtau,
                        op0=ALU.mult, op1=ALU.add,
                    )
                # ---- final p, normalize ----
                S_buf = smalls.tile([128, NT], F32, tag="Sbuf")
                for nt in range(NT):
                    r = rpool.tile([128, 1024], F32, tag="r")
                    nc.scalar.activation(
                        out=r, in_=xa_tiles[nt], func=AF.Relu,
                        bias=negtau[:, nt:nt + 1], scale=1.0,
                    )
                    nc.vector.scalar_tensor_tensor(
                        out=xa_tiles[nt], in0=r, scalar=1.0, in1=r,
                        op0=ALU.mult, op1=ALU.mult,
                        accum_out=S_buf[:, nt:nt + 1],
                    )
                recipS = smalls.tile([128, NT], F32, tag="recipS")
                nc.vector.reciprocal(out=recipS, in_=S_buf)
                for nt in range(NT):
                    nc.scalar.activation(
                        out=xa_tiles[nt], in_=xa_tiles[nt], func=AF.Copy,
                        bias=0.0, scale=recipS[:, nt:nt + 1],
                    )
                    nc.sync.dma_start(
                        out=attn_d[b, hh, nt * 128:(nt + 1) * 128, :],
                        in_=xa_tiles[nt],
                    )
                # ---- attn @ v (transpose attn per m-chunk, accumulate) ----
                avp = psav.tile([64, 1024], F32, tag="avp")
                for mt in range(8):
                    attnT = atp.tile([128, 1024], F32, tag="attnT")
                    for ng in range(2):
                        tp = psum.tile([128, 4, 128], F32, tag="half")
                        for j in range(4):
                            nt = ng * 4 + j
                            nc.tensor.transpose(
                                tp[:, j, :],
                                xa_tiles[nt][:, mt * 128:(mt + 1) * 128],
                                ident,
                            )
                        if ng == 0:
                            nc.scalar.copy(
                                out=attnT[:, ng * 512:(ng + 1) * 512], in_=tp
                            )
                        else:
                            nc.vector.tensor_copy(
                                out=attnT[:, ng * 512:(ng + 1) * 512], in_=tp
                            )
                    for nh in range(2):
                        nc.tensor.matmul(
                            avp[:, nh * 512:(nh + 1) * 512],
                            vt[:, mt, dlo:dlo + 64],
                            attnT[:, nh * 512:(nh + 1) * 512],
                            start=(mt == 0),
                            stop=(mt == 7),
                        )
                nc.scalar.activation(
                    out=outT_h[hh][:, b, :], in_=avp, func=AF.Identity,
                    bias=bv_h[hh][:, 0:1], scale=1.0,
                )
            # ---- output projection for batch b (both heads accumulate) ----
            for nt in range(NT):
                op = psum.tile([128, 1024], F32, tag="wide")
                for co in range(2):
                    for hh in range(HPC):
                        nc.tensor.matmul(
                            op[:, co * 512:(co + 1) * 512],
                            outT_h[hh][:, b, nt * 128:(nt + 1) * 128],
                            wo_h[hh][:, co * 512:(co + 1) * 512],
                            start=(hh == 0),
                            stop=(hh == HPC - 1),
                        )
                oe = opool.tile([128, 1024], F32, tag="oev")
                nc.scalar.copy(out=oe, in_=op)
                nc.sync.dma_start(
                    out=out_d[b, nt * 128:(nt + 1) * 128, :], in_=oe
                )
    return nc


_NC_CACHE = None


def _get_nc():
    global _NC_CACHE
    if _NC_CACHE is None:
        _NC_CACHE = _build_kernel()
    return _NC_CACHE


def kernel(x, Wq, bq, Wk, bk, Wv, bv, Wo, bo, _profile=None):
    from concourse.bass_utils import run_bass_kernel_spmd

    x = np.ascontiguousarray(np.asarray(x, dtype=np.float32))
    nc = _get_nc()
    in_maps = []
    for c in range(NCORES):
        cols = slice(c * HPC * DH, (c + 1) * HPC * DH)
        in_maps.append({
            "x": x,
            "wq": np.ascontiguousarray(np.asarray(Wq, np.float32)[:, cols]),
            "wk": np.ascontiguousarray(np.asarray(Wk, np.float32)[:, cols]),
            "wv": np.ascontiguousarray(np.asarray(Wv, np.float32)[:, cols]),
            "wo": np.ascontiguousarray(np.asarray(Wo, np.float32)[cols, :]),
            "bq": np.ascontiguousarray(np.asarray(bq, np.float32)[cols, None]),
            "bk": np.ascontiguousarray(np.asarray(bk, np.float32)[cols, None]),
            "bv": np.ascontiguousarray(np.asarray(bv, np.float32)[cols, None]),
        })
    res = run_bass_kernel_spmd(nc, in_maps, core_ids=list(range(NCORES)))
    attn = np.empty((B, H, N, N), np.float32)
    out = np.zeros((B, N, C), np.float64)
    for c in range(NCORES):
        attn[:, c * HPC:(c + 1) * HPC] = res.results[c]["attn_out"]
        out += res.results[c]["out_part"]
    out = (out + np.asarray(bo, np.float64)).astype(np.float32)
    kernel._last_results = res
    return out, attn
